# revision 26
# baseline (speedup 1.0000x reference)
"""Trainium2 Bass kernel for ActiveMatter NPINN PDE loss (plan Omega).

Computes (total, loss_cont, loss_conc, loss_dxx) over u, v, c, Dxx of shape
(4, 22, 256, 256) fp32.

Sharding: 8 cores, core i <- (batch b = i//2, time-half h = i%2), 10 interior
frames each; c/Dxx carry a +-1 frame halo.  Per-core partial sums of
(div'^2, T_c^2, T_d^2) with T = 2*DX*R are combined on the host.

v3 design:
 - every stencil / shifted-pair combination runs on the PE as an fp8e4
   DoubleRow matmul: 2 k-tiles per instruction (block pair for y-stencils
   with wrap correction, shifted-view pairs for x/t packs) at 0.5 cyc/row.
 - all fp8 weight ratios are exact (laplacian rides a 1.28-prescaled host
   copy so its taps are -2/+8/-2).
 - products (upwind selects, f*div) are 1x PSUM-coupled DVE/Pool ops that
   write fp8 directly (free, PSUM reads are 1x anyway); squares accumulate
   on Act straight from PSUM.
 - host pre-packs every tensor into the exact SBUF layout (128-partition,
   halo cols included) so each DMA is one contiguous run per partition.

Layout: frame rows r = 128*j + p -> [128 part, 2 blk, w]; x-halo cols
w~ = k <-> col (k-2) mod 256 (260 wide).
"""

import sys

for _p in ("/opt/trn_rl_repo",):
    if _p not in sys.path:
        sys.path.insert(0, _p)

import numpy as np

import concourse.bass as bass
import concourse.bacc as bacc
import concourse.mybir as mybir
from concourse.tile import TileContext
from concourse.bass_utils import run_bass_kernel_spmd

# ---------------------------------------------------------------- constants
B, T, H, W = 4, 22, 256, 256
N_CORES = 8
T_INT = 10
T_HALO = T_INT + 2
DX = 10.0 / 256.0
F32 = mybir.dt.float32
BF = mybir.dt.bfloat16
FP8 = mybir.dt.float8e4
AL = mybir.AluOpType
AF = mybir.ActivationFunctionType
DR = mybir.MatmulPerfMode.DoubleRow

_CACHE = {}


# ------------------------------------------------------- stencil matrices
def _circ_pair(taps: dict) -> np.ndarray:
    """[2, 128, 128] (main.T, corr.T) lhsT blocks for the periodic row
    stencil out[h] = sum_s taps[s] * f[(h+s) % 256], rows packed 2x128."""
    M = np.zeros((256, 256), np.float64)
    for s, a in taps.items():
        for h in range(256):
            M[h, (h + s) % 256] += a
    A = M[:128, :128]
    C = M[:128, 128:256]
    assert np.allclose(M[128:, 128:], A) and np.allclose(M[128:, :128], C)
    return np.stack([A.T, C.T]).astype(np.float32)


def _pair_diag(w0: float, w1: float) -> np.ndarray:
    I = np.eye(128, dtype=np.float32)
    return np.stack([w0 * I, w1 * I])


def _weights() -> np.ndarray:
    mats = [
        _circ_pair({1: 1.0, -1: -1.0}),    # 0 dvy-y
        _circ_pair({0: 1.0, 1: 1.0}),      # 1 vcn
        _circ_pair({0: 3.0, -1: -1.0}),    # 2 gp
        _circ_pair({1: 3.0, 2: -1.0}),     # 3 gm
        _circ_pair({0: 1.0, -1: -1.0}),    # 4 s5
        _circ_pair({1: -2.0, 0: 8.0, -1: -2.0}),  # 5 s4' (y-lap + x-lap centre)
        _pair_diag(1.0, -1.0),             # 6 (u+1,u-1) dvy-x / convx-p
        _pair_diag(-1.0, 1.0),             # 7 convx-q
        _pair_diag(1.0, 1.0),              # 8 uc2
        _pair_diag(3.0, -1.0),             # 9 gxp / gxm
        _pair_diag(0.15625, -0.15625),     # 10 dfdt (DX/DT)
        _pair_diag(-2.0, -2.0),            # 11 x-lap sides
        _pair_diag(-0.5, -0.5),            # 12 pd
    ]
    # [13, 2, 128, 128] (w, i, k, m) -> [128(k), 13, 2, 128(m)]
    return np.ascontiguousarray(np.transpose(np.stack(mats), (2, 0, 1, 3)))


NW = 13


# ------------------------------------------------------------ AP helpers
def _iview_swap(a3: bass.AP, j: int) -> bass.AP:
    """[128, 2(i), 256] ifmap for out-block j from a [128, 2, 256] frame
    view: i=0 -> block j, i=1 -> block 1-j."""
    a = [list(x) for x in a3.ap]
    assert len(a) == 3 and a[1][1] == 2
    sj = a[1][0]
    return bass.AP(tensor=a3.tensor, offset=a3.offset + j * sj,
                   ap=[a[0], [(1 - 2 * j) * sj, 2], a[2]])


def _pack2(v0: bass.AP, v1: bass.AP) -> bass.AP:
    """[128, 2(i), 2(j), 256] ifmap from two same-shape [128, 2, 256]
    views of one tensor (i-dim = the pack pair)."""
    assert v0.tensor is v1.tensor
    a = [list(x) for x in v0.ap]
    assert len(a) == 3 and a[1][1] == 2
    di = v1.offset - v0.offset
    return bass.AP(tensor=v0.tensor, offset=v0.offset,
                   ap=[a[0], [di, 2], a[1], a[2]])


# ------------------------------------------------------------ graph build
def _build():
    nc = bacc.Bacc("TRN2")
    u8e = nc.declare_dram_parameter("u8", [128, T_INT, 2, 260], FP8, isOutput=False)
    v8e = nc.declare_dram_parameter("v8", [128, T_INT, 2, 2, 256], FP8, isOutput=False)
    c8e = nc.declare_dram_parameter("c8", [128, T_HALO, 2, 2, 260], FP8, isOutput=False)
    d8e = nc.declare_dram_parameter("d8", [128, T_HALO, 2, 2, 260], FP8, isOutput=False)
    cbe = nc.declare_dram_parameter("c8b", [128, T_INT, 2, 2, 260], FP8, isOutput=False)
    dbe = nc.declare_dram_parameter("d8b", [128, T_INT, 2, 2, 260], FP8, isOutput=False)
    swe = nc.declare_dram_parameter("stw", [128, NW, 2, 128], FP8, isOutput=False)
    out_ext = nc.declare_dram_parameter("out", [1, 4], F32, isOutput=True)

    with TileContext(nc) as tc:
        with (
            tc.tile_pool(name="const", bufs=1) as constp,
            tc.tile_pool(name="frames", bufs=1) as framesp,
            tc.tile_pool(name="scr", bufs=12) as scr,
            tc.tile_pool(name="psum", bufs=1, space="PSUM") as psp,
        ):
            stw = constp.tile([128, NW, 2, 128], FP8, name="stw_sb")
            nc.sync.dma_start(out=stw, in_=swe[:, :, :, :])

            ones = constp.tile([128, 1], F32, name="ones_sb")
            nc.vector.memset(ones, 1.0)

            accs = constp.tile([128, 3, T_INT], F32, name="accs")

            u8 = framesp.tile([128, T_INT, 2, 260], FP8, name="u8")
            v8 = framesp.tile([128, T_INT, 2, 2, 256], FP8, name="v8")
            c8 = framesp.tile([128, T_HALO, 2, 2, 260], FP8, name="c8")
            d8 = framesp.tile([128, T_HALO, 2, 2, 260], FP8, name="d8")
            c8b = framesp.tile([128, T_INT, 2, 2, 260], FP8, name="c8b")
            d8b = framesp.tile([128, T_INT, 2, 2, 260], FP8, name="d8b")
            ucpM = framesp.tile([128, T_INT, 2, 256], BF, name="ucpM")
            ucmM = framesp.tile([128, T_INT, 2, 256], BF, name="ucmM")
            vcnM = framesp.tile([128, T_INT, 2, 256], BF, name="vcnM")
            divM = framesp.tile([128, T_INT, 2, 256], BF, name="divM")
            sqd = framesp.tile([128, T_INT, 2, 256], BF, name="sqd")

            # Input DMA split across the three DGE-capable queues
            # (Sync / Act / Pool) in consumption order, so wave-1 transfers
            # parallelize and the first matmuls unblock in ~3 us.
            nc.sync.dma_start(out=u8[:, 0:1], in_=u8e[:, 0:1])
            nc.sync.dma_start(out=v8[:, 0:1], in_=v8e[:, 0:1])
            nc.scalar.dma_start(out=c8[:, 0:3], in_=c8e[:, 0:3])
            nc.scalar.dma_start(out=d8[:, 0:3], in_=d8e[:, 0:3])
            nc.scalar.dma_start(out=c8b[:, 0:1], in_=cbe[:, 0:1])
            nc.scalar.dma_start(out=d8b[:, 0:1], in_=dbe[:, 0:1])
            nc.sync.dma_start(out=v8[:, 1:5], in_=v8e[:, 1:5])
            nc.sync.dma_start(out=u8[:, 1:5], in_=u8e[:, 1:5])
            nc.sync.dma_start(out=c8[:, 3:8], in_=c8e[:, 3:8])
            nc.sync.dma_start(out=c8b[:, 1:5], in_=cbe[:, 1:5])
            nc.scalar.dma_start(out=d8[:, 3:8], in_=d8e[:, 3:8])
            nc.scalar.dma_start(out=d8b[:, 1:5], in_=dbe[:, 1:5])
            nc.sync.dma_start(out=u8[:, 5:10], in_=u8e[:, 5:10])
            nc.sync.dma_start(out=v8[:, 5:10], in_=v8e[:, 5:10])
            nc.gpsimd.dma_start(out=c8[:, 8:12], in_=c8e[:, 8:12])
            nc.gpsimd.dma_start(out=d8[:, 8:12], in_=d8e[:, 8:12])
            nc.gpsimd.dma_start(out=c8b[:, 5:10], in_=cbe[:, 5:10])
            nc.gpsimd.dma_start(out=d8b[:, 5:10], in_=dbe[:, 5:10])

            def ev(tile, t, k=0):
                # even frame view [128, 2, 256] at col offset k (halo'd tile)
                return tile[:, t, :, 2 + k:258 + k]

            def evd(tile, t, k=0):
                # normal-copy frame view of a dup-swapped tile
                return tile[:, t, 0, :, 2 + k:258 + k]

            def stencil(ps3, w, rhs3, start, stop):
                # y-stencil DR pair: out_j = main @ f_j + corr @ f_{1-j}
                for j in (0, 1):
                    nc.tensor.matmul(ps3[:, j, :], stw[:, w],
                                     _iview_swap(rhs3, j),
                                     start=start and j == 0,
                                     stop=stop and j == 1,
                                     skip_group_check=True,
                                     perf_mode=DR)

            def stencil_h(ps3, w, tile, t, start, stop, k=0, halo=True):
                # y-stencil on a host-dup tile: one DR matmul, ifmap
                # [p][v=(normal,swapped)][j][col]
                c0 = 2 + k if halo else 0
                nc.tensor.matmul(ps3[:, :, :], stw[:, w],
                                 tile[:, t, :, :, c0:c0 + 256],
                                 start=start, stop=stop,
                                 skip_group_check=True, perf_mode=DR)

            def pack(ps3, w, v0, v1, start, stop):
                # x/t identity pack: out = w0 * v0 + w1 * v1 (N=512 DR)
                nc.tensor.matmul(ps3[:, :, :], stw[:, w], _pack2(v0, v1),
                                 start=start, stop=stop,
                                 skip_group_check=True, perf_mode=DR)

            STT = nc.vector.scalar_tensor_tensor
            TT = nc.vector.tensor_tensor

            def shared(t):
                uc2p = psp.tile([128, 2, 256], F32, name="uc2p", tag="aux",
                                bufs=3)
                pack(uc2p, 8, ev(u8, t, 0), ev(u8, t, 1), True, True)
                nc.scalar.activation(ucpM[:, t], uc2p, AF.Relu, scale=0.5)
                nc.scalar.activation(ucmM[:, t], uc2p, AF.Relu, scale=-0.5)

                vcnp = psp.tile([128, 2, 256], F32, name="vcnp", tag="aux",
                                bufs=3)
                stencil_h(vcnp, 1, v8, t, True, True, halo=False)
                nc.scalar.mul(vcnM[:, t], vcnp, 0.5)

                dvyp = psp.tile([128, 2, 256], F32, name="dvyp", tag="aux",
                                bufs=3)
                stencil_h(dvyp, 0, v8, t, True, False, halo=False)
                pack(dvyp, 6, ev(u8, t, 1), ev(u8, t, -1), False, True)
                nc.scalar.copy(divM[:, t], dvyp)

            def fields(t):
                for fi, (f8, fb8) in enumerate(((c8, c8b), (d8, d8b))):
                    sfx = f"{'cd'[fi]}{t}"
                    gp = psp.tile([128, 2, 256], F32, name="gp", tag="g",
                                  bufs=3)
                    stencil_h(gp, 2, f8, t + 1, True, True)
                    gm = psp.tile([128, 2, 256], F32, name="gm", tag="g",
                                  bufs=3)
                    stencil_h(gm, 3, f8, t + 1, True, True)
                    p28 = scr.tile([128, 2, 256], FP8, name="p28", tag="p2")
                    STT(p28, vcnM[:, t], 0.0, gp, AL.max, AL.mult)
                    q28 = scr.tile([128, 2, 256], FP8, name="q28", tag="q2")
                    STT(q28, vcnM[:, t], 0.0, gm, AL.min, AL.mult)

                    gxp = psp.tile([128, 2, 256], F32, name="gxp", tag="g",
                                   bufs=3)
                    pack(gxp, 9, evd(f8, t + 1, 0), evd(f8, t + 1, -1),
                         True, True)
                    gxm = psp.tile([128, 2, 256], F32, name="gxm", tag="g",
                                   bufs=3)
                    pack(gxm, 9, evd(f8, t + 1, 1), evd(f8, t + 1, 2),
                         True, True)
                    p8 = scr.tile([128, 2, 258], FP8, name="p8", tag="p8")
                    TT(p8[:, :, 1:257], ucpM[:, t], gxp, AL.mult)
                    nc.gpsimd.tensor_copy(p8[:, :, 0:1], p8[:, :, 256:257])
                    q8 = scr.tile([128, 2, 258], FP8, name="q8", tag="q8")
                    TT(q8[:, :, 1:257], ucmM[:, t], gxm, AL.mult)
                    nc.gpsimd.tensor_copy(q8[:, :, 0:1], q8[:, :, 256:257])

                    pd8 = scr.tile([128, 2, 256], FP8, name="pd8", tag="pd")
                    nc.gpsimd.tensor_tensor(pd8, evd(f8, t + 1), divM[:, t],
                                            AL.mult)

                    t2p = psp.tile([128, 2, 256], F32, name="t2p", tag="t2",
                                   bufs=2)
                    pack(t2p, 10, evd(f8, t + 2), evd(f8, t), True, False)
                    stencil_h(t2p, 5, fb8, t, False, False)
                    pack(t2p, 11, evd(fb8, t, 1), evd(fb8, t, -1),
                         False, False)
                    stencil(t2p, 4, p28[:, :, :], False, False)
                    stencil(t2p, 4, q28[:, :, :], False, False)
                    pack(t2p, 6, p8[:, :, 1:257], p8[:, :, 0:256],
                         False, False)
                    pack(t2p, 7, q8[:, :, 1:257], q8[:, :, 0:256],
                         False, False)
                    pack(t2p, 12, pd8[:, :, :], pd8[:, :, :], False, True)
                    sq = scr.tile([128, 2, 256], BF, name="sq", tag="sq")
                    nc.scalar.activation(sq, t2p, AF.Square,
                                         accum_out=accs[:, 1 + fi, t:t + 1])
                if t % 2 == 1:
                    # continuity loss for the frame pair (t-1, t)
                    k = t // 2
                    nc.scalar.activation(
                        sqd[:, t - 1:t + 1].rearrange("p t j x -> p (t j x)"),
                        divM[:, t - 1:t + 1].rearrange("p t j x -> p (t j x)"),
                        AF.Square, accum_out=accs[:, 0, k:k + 1])

            # shared chain runs one frame ahead of the field chains
            shared(0)
            for t in range(T_INT):
                if t + 1 < T_INT:
                    shared(t + 1)
                fields(t)

            # ---------------- final reduction to [1, 3]
            red3 = constp.tile([128, 3], F32, name="red3")
            nc.vector.tensor_reduce(red3[:, 0:1], accs[:, 0, 0:5],
                                    mybir.AxisListType.X, AL.add)
            for k in (1, 2):
                nc.vector.tensor_reduce(red3[:, k:k + 1], accs[:, k, :],
                                        mybir.AxisListType.X, AL.add)
            psr = psp.tile([1, 4], F32, name="psr", tag="aux", bufs=3)
            nc.tensor.matmul(psr[:, 0:3], ones, red3, start=True, stop=True)
            outt = constp.tile([1, 4], F32, name="outt")
            nc.vector.memset(outt, 0.0)
            nc.scalar.copy(outt[:, 0:3], psr[:, 0:3])
            nc.sync.dma_start(out=out_ext[:, :], in_=outt)

    nc.compile()
    return nc


def _get_nc():
    if "nc" not in _CACHE:
        _CACHE["nc"] = _build()
        _CACHE["stw"] = _weights()
    return _CACHE["nc"]


def _prepack(x, halo, dup=False):
    """[T', 256, 256] fp32 -> fp8 [128, T', (2,) 2, 260 or 256] SBUF layout.

    dup=True adds a block-swapped copy (dim 2) so y-stencils with wrap
    correction are a single DoubleRow matmul over an affine ifmap."""
    import ml_dtypes
    t = x.shape[0]
    y = x.reshape(t, 2, 128, 256).transpose(2, 0, 1, 3)
    if halo:
        y = np.concatenate([y[..., 254:256], y, y[..., 0:2]], axis=-1)
    if dup:
        y = np.stack([y, y[:, :, ::-1]], axis=2)
    return np.ascontiguousarray(y.astype(ml_dtypes.float8_e4m3))


def _make_in_maps(u, v, c, Dxx):
    import ml_dtypes
    u = np.asarray(u, dtype=np.float32)
    v = np.asarray(v, dtype=np.float32)
    c = np.asarray(c, dtype=np.float32)
    d = np.asarray(Dxx, dtype=np.float32)
    stw = np.ascontiguousarray(_CACHE["stw"].astype(ml_dtypes.float8_e4m3))
    in_maps = []
    for i in range(N_CORES):
        b, h = i // 2, i % 2
        t0 = 1 + T_INT * h
        in_maps.append({
            "u8": _prepack(u[b, t0:t0 + T_INT], True),
            "v8": _prepack(v[b, t0:t0 + T_INT], False, dup=True),
            "c8": _prepack(c[b, t0 - 1:t0 + T_INT + 1], True, dup=True),
            "d8": _prepack(d[b, t0 - 1:t0 + T_INT + 1], True, dup=True),
            "c8b": _prepack(1.28 * c[b, t0:t0 + T_INT], True, dup=True),
            "d8b": _prepack(1.28 * d[b, t0:t0 + T_INT], True, dup=True),
            "stw": stw,
        })
    return in_maps


def _combine(results):
    s = np.zeros(3, dtype=np.float64)
    for r in results:
        s += np.asarray(r["out"], dtype=np.float64)[0, :3]
    n = B * (T - 2) * H * W
    scale = 1.0 / (4.0 * DX * DX * n)
    loss_cont = scale * s[0]
    loss_conc = scale * s[1]
    loss_dxx = scale * s[2]
    total = loss_cont + loss_conc + loss_dxx
    return np.array([total, loss_cont, loss_conc, loss_dxx], dtype=np.float32)


def kernel(u, v, c, Dxx):
    nc = _get_nc()
    in_maps = _make_in_maps(u, v, c, Dxx)
    last_err = None
    for _attempt in range(3):
        try:
            res = run_bass_kernel_spmd(nc, in_maps,
                                       core_ids=list(range(N_CORES)))
            return _combine(res.results)
        except Exception as e:  # transient NRT/device hiccups: retry
            last_err = e
    raise last_err


if __name__ == "__main__":
    rng = np.random.default_rng(0)
    inputs = {
        "u": rng.standard_normal((B, T, H, W), dtype=np.float32),
        "v": rng.standard_normal((B, T, H, W), dtype=np.float32),
        "c": rng.random((B, T, H, W), dtype=np.float32),
        "Dxx": rng.random((B, T, H, W), dtype=np.float32),
    }
    print(kernel(**inputs))


# revision 28
# speedup vs baseline: 1.0898x; 1.0898x over previous
"""Trainium2 Bass kernel for ActiveMatter NPINN PDE loss (plan Omega).

Computes (total, loss_cont, loss_conc, loss_dxx) over u, v, c, Dxx of shape
(4, 22, 256, 256) fp32.

Sharding: 8 cores, core i <- (batch b = i//2, time-half h = i%2), 10 interior
frames each; c/Dxx carry a +-1 frame halo.  Per-core partial sums of
(div'^2, T_c^2, T_d^2) with T = 2*DX*R are combined on the host.

v3 design:
 - every stencil / shifted-pair combination runs on the PE as an fp8e4
   DoubleRow matmul: 2 k-tiles per instruction (block pair for y-stencils
   with wrap correction, shifted-view pairs for x/t packs) at 0.5 cyc/row.
 - all fp8 weight ratios are exact (laplacian rides a 1.28-prescaled host
   copy so its taps are -2/+8/-2).
 - products (upwind selects, f*div) are 1x PSUM-coupled DVE/Pool ops that
   write fp8 directly (free, PSUM reads are 1x anyway); squares accumulate
   on Act straight from PSUM.
 - host pre-packs every tensor into the exact SBUF layout (128-partition,
   halo cols included) so each DMA is one contiguous run per partition.

Layout: frame rows r = 128*j + p -> [128 part, 2 blk, w]; x-halo cols
w~ = k <-> col (k-2) mod 256 (260 wide).
"""

import sys

for _p in ("/opt/trn_rl_repo",):
    if _p not in sys.path:
        sys.path.insert(0, _p)

import numpy as np

import concourse.bass as bass
import concourse.bacc as bacc
import concourse.mybir as mybir
from concourse.tile import TileContext
from concourse.bass_utils import run_bass_kernel_spmd

# ---------------------------------------------------------------- constants
B, T, H, W = 4, 22, 256, 256
N_CORES = 8
T_INT = 10
T_HALO = T_INT + 2
DX = 10.0 / 256.0
F32 = mybir.dt.float32
BF = mybir.dt.bfloat16
FP8 = mybir.dt.float8e4
AL = mybir.AluOpType
AF = mybir.ActivationFunctionType
DR = mybir.MatmulPerfMode.DoubleRow

_CACHE = {}


# ------------------------------------------------------- stencil matrices
def _circ_pair(taps: dict) -> np.ndarray:
    """[2, 128, 128] (main.T, corr.T) lhsT blocks for the periodic row
    stencil out[h] = sum_s taps[s] * f[(h+s) % 256], rows packed 2x128."""
    M = np.zeros((256, 256), np.float64)
    for s, a in taps.items():
        for h in range(256):
            M[h, (h + s) % 256] += a
    A = M[:128, :128]
    C = M[:128, 128:256]
    assert np.allclose(M[128:, 128:], A) and np.allclose(M[128:, :128], C)
    return np.stack([A.T, C.T]).astype(np.float32)


def _pair_diag(w0: float, w1: float) -> np.ndarray:
    I = np.eye(128, dtype=np.float32)
    return np.stack([w0 * I, w1 * I])


def _weights() -> np.ndarray:
    mats = [
        _circ_pair({1: 1.0, -1: -1.0}),    # 0 dvy-y
        _circ_pair({0: 1.0, 1: 1.0}),      # 1 vcn
        _circ_pair({0: 3.0, -1: -1.0}),    # 2 gp
        _circ_pair({1: 3.0, 2: -1.0}),     # 3 gm
        _circ_pair({0: 1.0, -1: -1.0}),    # 4 s5
        _circ_pair({1: -2.0, 0: 8.0, -1: -2.0}),  # 5 s4' (y-lap + x-lap centre)
        _pair_diag(1.0, -1.0),             # 6 (u+1,u-1) dvy-x / convx-p
        _pair_diag(-1.0, 1.0),             # 7 convx-q
        _pair_diag(1.0, 1.0),              # 8 uc2
        _pair_diag(3.0, -1.0),             # 9 gxp / gxm
        _pair_diag(0.15625, -0.15625),     # 10 dfdt (DX/DT)
        _pair_diag(-2.0, -2.0),            # 11 x-lap sides
        _pair_diag(-0.5, -0.5),            # 12 pd
    ]
    # [13, 2, 128, 128] (w, i, k, m) -> [128(k), 13, 2, 128(m)]
    return np.ascontiguousarray(np.transpose(np.stack(mats), (2, 0, 1, 3)))


NW = 13


# ------------------------------------------------------------ AP helpers
def _iview_swap(a3: bass.AP, j: int) -> bass.AP:
    """[128, 2(i), 256] ifmap for out-block j from a [128, 2, 256] frame
    view: i=0 -> block j, i=1 -> block 1-j."""
    a = [list(x) for x in a3.ap]
    assert len(a) == 3 and a[1][1] == 2
    sj = a[1][0]
    return bass.AP(tensor=a3.tensor, offset=a3.offset + j * sj,
                   ap=[a[0], [(1 - 2 * j) * sj, 2], a[2]])


def _pack2(v0: bass.AP, v1: bass.AP) -> bass.AP:
    """[128, 2(i), 2(j), 256] ifmap from two same-shape [128, 2, 256]
    views of one tensor (i-dim = the pack pair)."""
    assert v0.tensor is v1.tensor
    a = [list(x) for x in v0.ap]
    assert len(a) == 3 and a[1][1] == 2
    di = v1.offset - v0.offset
    return bass.AP(tensor=v0.tensor, offset=v0.offset,
                   ap=[a[0], [di, 2], a[1], a[2]])


# ------------------------------------------------------------ graph build
def _build():
    nc = bacc.Bacc("TRN2")
    u8e = nc.declare_dram_parameter("u8", [128, T_INT, 2, 260], FP8, isOutput=False)
    v8e = nc.declare_dram_parameter("v8", [128, T_INT, 2, 2, 256], FP8, isOutput=False)
    c8e = nc.declare_dram_parameter("c8", [128, T_HALO, 2, 2, 260], FP8, isOutput=False)
    d8e = nc.declare_dram_parameter("d8", [128, T_HALO, 2, 2, 260], FP8, isOutput=False)
    cbe = nc.declare_dram_parameter("c8b", [128, T_INT, 2, 2, 260], FP8, isOutput=False)
    dbe = nc.declare_dram_parameter("d8b", [128, T_INT, 2, 2, 260], FP8, isOutput=False)
    swe = nc.declare_dram_parameter("stw", [128, NW, 2, 128], FP8, isOutput=False)
    out_ext = nc.declare_dram_parameter("out", [128, 3, T_INT], F32,
                                        isOutput=True)

    with TileContext(nc) as tc:
        with (
            tc.tile_pool(name="const", bufs=1) as constp,
            tc.tile_pool(name="frames", bufs=1) as framesp,
            tc.tile_pool(name="scr", bufs=12) as scr,
            tc.tile_pool(name="psum", bufs=1, space="PSUM") as psp,
        ):
            stw = constp.tile([128, NW, 2, 128], FP8, name="stw_sb")
            nc.sync.dma_start(out=stw, in_=swe[:, :, :, :])

            accs = constp.tile([128, 3, T_INT], F32, name="accs")

            u8 = framesp.tile([128, T_INT, 2, 260], FP8, name="u8")
            v8 = framesp.tile([128, T_INT, 2, 2, 256], FP8, name="v8")
            c8 = framesp.tile([128, T_HALO, 2, 2, 260], FP8, name="c8")
            d8 = framesp.tile([128, T_HALO, 2, 2, 260], FP8, name="d8")
            c8b = framesp.tile([128, T_INT, 2, 2, 260], FP8, name="c8b")
            d8b = framesp.tile([128, T_INT, 2, 2, 260], FP8, name="d8b")
            ucpM = framesp.tile([128, T_INT, 2, 256], BF, name="ucpM")
            ucmM = framesp.tile([128, T_INT, 2, 256], BF, name="ucmM")
            vcnM = framesp.tile([128, T_INT, 2, 256], BF, name="vcnM")
            divM = framesp.tile([128, T_INT, 2, 256], BF, name="divM")
            sqd = framesp.tile([128, T_INT, 2, 256], BF, name="sqd")

            # DMA waves: tiny first wave so compute starts early
            WAVES = [(0, 1, 0, 3), (1, 4, 3, 5), (5, 5, 8, 4)]
            for (ti, ni, th, nh) in WAVES:
                nc.sync.dma_start(out=u8[:, ti:ti + ni], in_=u8e[:, ti:ti + ni])
                nc.sync.dma_start(out=v8[:, ti:ti + ni], in_=v8e[:, ti:ti + ni])
                nc.sync.dma_start(out=c8[:, th:th + nh], in_=c8e[:, th:th + nh])
                nc.sync.dma_start(out=d8[:, th:th + nh], in_=d8e[:, th:th + nh])
                nc.sync.dma_start(out=c8b[:, ti:ti + ni], in_=cbe[:, ti:ti + ni])
                nc.sync.dma_start(out=d8b[:, ti:ti + ni], in_=dbe[:, ti:ti + ni])

            def ev(tile, t, k=0):
                # even frame view [128, 2, 256] at col offset k (halo'd tile)
                return tile[:, t, :, 2 + k:258 + k]

            def evd(tile, t, k=0):
                # normal-copy frame view of a dup-swapped tile
                return tile[:, t, 0, :, 2 + k:258 + k]

            def stencil(ps3, w, rhs3, start, stop):
                # y-stencil DR pair: out_j = main @ f_j + corr @ f_{1-j}
                for j in (0, 1):
                    nc.tensor.matmul(ps3[:, j, :], stw[:, w],
                                     _iview_swap(rhs3, j),
                                     start=start and j == 0,
                                     stop=stop and j == 1,
                                     skip_group_check=True,
                                     perf_mode=DR)

            def stencil_h(ps3, w, tile, t, start, stop, k=0, halo=True):
                # y-stencil on a host-dup tile: one DR matmul, ifmap
                # [p][v=(normal,swapped)][j][col]
                c0 = 2 + k if halo else 0
                nc.tensor.matmul(ps3[:, :, :], stw[:, w],
                                 tile[:, t, :, :, c0:c0 + 256],
                                 start=start, stop=stop,
                                 skip_group_check=True, perf_mode=DR)

            def pack(ps3, w, v0, v1, start, stop):
                # x/t identity pack: out = w0 * v0 + w1 * v1 (N=512 DR)
                nc.tensor.matmul(ps3[:, :, :], stw[:, w], _pack2(v0, v1),
                                 start=start, stop=stop,
                                 skip_group_check=True, perf_mode=DR)

            STT = nc.vector.scalar_tensor_tensor
            TT = nc.vector.tensor_tensor

            def shared(t):
                uc2p = psp.tile([128, 2, 256], F32, name="uc2p", tag="aux",
                                bufs=3)
                pack(uc2p, 8, ev(u8, t, 0), ev(u8, t, 1), True, True)
                nc.scalar.activation(ucpM[:, t], uc2p, AF.Relu, scale=0.5)
                nc.scalar.activation(ucmM[:, t], uc2p, AF.Relu, scale=-0.5)

                vcnp = psp.tile([128, 2, 256], F32, name="vcnp", tag="aux",
                                bufs=3)
                stencil_h(vcnp, 1, v8, t, True, True, halo=False)
                nc.scalar.mul(vcnM[:, t], vcnp, 0.5)

                dvyp = psp.tile([128, 2, 256], F32, name="dvyp", tag="aux",
                                bufs=3)
                stencil_h(dvyp, 0, v8, t, True, False, halo=False)
                pack(dvyp, 6, ev(u8, t, 1), ev(u8, t, -1), False, True)
                nc.scalar.copy(divM[:, t], dvyp)

            def fields(t):
                for fi, (f8, fb8) in enumerate(((c8, c8b), (d8, d8b))):
                    sfx = f"{'cd'[fi]}{t}"
                    gp = psp.tile([128, 2, 256], F32, name="gp", tag="g",
                                  bufs=3)
                    stencil_h(gp, 2, f8, t + 1, True, True)
                    gm = psp.tile([128, 2, 256], F32, name="gm", tag="g",
                                  bufs=3)
                    stencil_h(gm, 3, f8, t + 1, True, True)
                    p28 = scr.tile([128, 2, 256], FP8, name="p28", tag="p2")
                    STT(p28, vcnM[:, t], 0.0, gp, AL.max, AL.mult)
                    q28 = scr.tile([128, 2, 256], FP8, name="q28", tag="q2")
                    STT(q28, vcnM[:, t], 0.0, gm, AL.min, AL.mult)

                    gxp = psp.tile([128, 2, 256], F32, name="gxp", tag="g",
                                   bufs=3)
                    pack(gxp, 9, evd(f8, t + 1, 0), evd(f8, t + 1, -1),
                         True, True)
                    gxm = psp.tile([128, 2, 256], F32, name="gxm", tag="g",
                                   bufs=3)
                    pack(gxm, 9, evd(f8, t + 1, 1), evd(f8, t + 1, 2),
                         True, True)
                    p8 = scr.tile([128, 2, 258], FP8, name="p8", tag="p8")
                    TT(p8[:, :, 1:257], ucpM[:, t], gxp, AL.mult)
                    nc.gpsimd.tensor_copy(p8[:, :, 0:1], p8[:, :, 256:257])
                    q8 = scr.tile([128, 2, 258], FP8, name="q8", tag="q8")
                    TT(q8[:, :, 1:257], ucmM[:, t], gxm, AL.mult)
                    nc.gpsimd.tensor_copy(q8[:, :, 0:1], q8[:, :, 256:257])

                    pd8 = scr.tile([128, 2, 256], FP8, name="pd8", tag="pd")
                    nc.gpsimd.tensor_tensor(pd8, evd(f8, t + 1), divM[:, t],
                                            AL.mult)

                    t2p = psp.tile([128, 2, 256], F32, name="t2p", tag="t2",
                                   bufs=2)
                    pack(t2p, 10, evd(f8, t + 2), evd(f8, t), True, False)
                    stencil_h(t2p, 5, fb8, t, False, False)
                    pack(t2p, 11, evd(fb8, t, 1), evd(fb8, t, -1),
                         False, False)
                    stencil(t2p, 4, p28[:, :, :], False, False)
                    stencil(t2p, 4, q28[:, :, :], False, False)
                    pack(t2p, 6, p8[:, :, 1:257], p8[:, :, 0:256],
                         False, False)
                    pack(t2p, 7, q8[:, :, 1:257], q8[:, :, 0:256],
                         False, False)
                    pack(t2p, 12, pd8[:, :, :], pd8[:, :, :], False, True)
                    sq = scr.tile([128, 2, 256], BF, name="sq", tag="sq")
                    nc.scalar.activation(sq, t2p, AF.Square,
                                         accum_out=accs[:, 1 + fi, t:t + 1])
                if t % 2 == 1:
                    # continuity loss for the frame pair (t-1, t)
                    k = t // 2
                    nc.scalar.activation(
                        sqd[:, t - 1:t + 1].rearrange("p t j x -> p (t j x)"),
                        divM[:, t - 1:t + 1].rearrange("p t j x -> p (t j x)"),
                        AF.Square, accum_out=accs[:, 0, k:k + 1])

            # shared chain runs one frame ahead of the field chains
            shared(0)
            for t in range(T_INT):
                if t + 1 < T_INT:
                    shared(t + 1)
                fields(t)

            # raw per-partition accumulators out; host does the final sum
            nc.sync.dma_start(out=out_ext[:, :, :], in_=accs)

    nc.compile()
    return nc


def _get_nc():
    if "nc" not in _CACHE:
        _CACHE["nc"] = _build()
        _CACHE["stw"] = _weights()
    return _CACHE["nc"]


def _prepack(x, halo, dup=False):
    """[T', 256, 256] fp32 -> fp8 [128, T', (2,) 2, 260 or 256] SBUF layout.

    dup=True adds a block-swapped copy (dim 2) so y-stencils with wrap
    correction are a single DoubleRow matmul over an affine ifmap."""
    import ml_dtypes
    t = x.shape[0]
    y = x.reshape(t, 2, 128, 256).transpose(2, 0, 1, 3)
    if halo:
        y = np.concatenate([y[..., 254:256], y, y[..., 0:2]], axis=-1)
    if dup:
        y = np.stack([y, y[:, :, ::-1]], axis=2)
    return np.ascontiguousarray(y.astype(ml_dtypes.float8_e4m3))


def _make_in_maps(u, v, c, Dxx):
    import ml_dtypes
    u = np.asarray(u, dtype=np.float32)
    v = np.asarray(v, dtype=np.float32)
    c = np.asarray(c, dtype=np.float32)
    d = np.asarray(Dxx, dtype=np.float32)
    stw = np.ascontiguousarray(_CACHE["stw"].astype(ml_dtypes.float8_e4m3))
    in_maps = []
    for i in range(N_CORES):
        b, h = i // 2, i % 2
        t0 = 1 + T_INT * h
        in_maps.append({
            "u8": _prepack(u[b, t0:t0 + T_INT], True),
            "v8": _prepack(v[b, t0:t0 + T_INT], False, dup=True),
            "c8": _prepack(c[b, t0 - 1:t0 + T_INT + 1], True, dup=True),
            "d8": _prepack(d[b, t0 - 1:t0 + T_INT + 1], True, dup=True),
            "c8b": _prepack(1.28 * c[b, t0:t0 + T_INT], True, dup=True),
            "d8b": _prepack(1.28 * d[b, t0:t0 + T_INT], True, dup=True),
            "stw": stw,
        })
    return in_maps


def _combine(results):
    s = np.zeros(3, dtype=np.float64)
    for r in results:
        a = np.asarray(r["out"], dtype=np.float64)
        s[0] += a[:, 0, 0:5].sum()
        s[1] += a[:, 1, :].sum()
        s[2] += a[:, 2, :].sum()
    n = B * (T - 2) * H * W
    scale = 1.0 / (4.0 * DX * DX * n)
    loss_cont = scale * s[0]
    loss_conc = scale * s[1]
    loss_dxx = scale * s[2]
    total = loss_cont + loss_conc + loss_dxx
    return np.array([total, loss_cont, loss_conc, loss_dxx], dtype=np.float32)


def kernel(u, v, c, Dxx):
    nc = _get_nc()
    in_maps = _make_in_maps(u, v, c, Dxx)
    last_err = None
    for _attempt in range(3):
        try:
            res = run_bass_kernel_spmd(nc, in_maps,
                                       core_ids=list(range(N_CORES)))
            return _combine(res.results)
        except Exception as e:  # transient NRT/device hiccups: retry
            last_err = e
    raise last_err


if __name__ == "__main__":
    rng = np.random.default_rng(0)
    inputs = {
        "u": rng.standard_normal((B, T, H, W), dtype=np.float32),
        "v": rng.standard_normal((B, T, H, W), dtype=np.float32),
        "c": rng.random((B, T, H, W), dtype=np.float32),
        "Dxx": rng.random((B, T, H, W), dtype=np.float32),
    }
    print(kernel(**inputs))


# revision 29
# speedup vs baseline: 1.2835x; 1.1777x over previous
"""Trainium2 Bass kernel for ActiveMatter NPINN PDE loss (plan Omega).

Computes (total, loss_cont, loss_conc, loss_dxx) over u, v, c, Dxx of shape
(4, 22, 256, 256) fp32.

Sharding: 8 cores, core i <- (batch b = i//2, time-half h = i%2), 10 interior
frames each; c/Dxx carry a +-1 frame halo.  Per-core partial sums of
(div'^2, T_c^2, T_d^2) with T = 2*DX*R are combined on the host.

v3 design:
 - every stencil / shifted-pair combination runs on the PE as an fp8e4
   DoubleRow matmul: 2 k-tiles per instruction (block pair for y-stencils
   with wrap correction, shifted-view pairs for x/t packs) at 0.5 cyc/row.
 - all fp8 weight ratios are exact (laplacian rides a 1.28-prescaled host
   copy so its taps are -2/+8/-2).
 - products (upwind selects, f*div) are 1x PSUM-coupled DVE/Pool ops that
   write fp8 directly (free, PSUM reads are 1x anyway); squares accumulate
   on Act straight from PSUM.
 - host pre-packs every tensor into the exact SBUF layout (128-partition,
   halo cols included) so each DMA is one contiguous run per partition.

Layout: frame rows r = 128*j + p -> [128 part, 2 blk, w]; x-halo cols
w~ = k <-> col (k-2) mod 256 (260 wide).
"""

import sys

for _p in ("/opt/trn_rl_repo",):
    if _p not in sys.path:
        sys.path.insert(0, _p)

import numpy as np

import concourse.bass as bass
import concourse.bacc as bacc
import concourse.mybir as mybir
from concourse.tile import TileContext
from concourse.bass_utils import run_bass_kernel_spmd

# ---------------------------------------------------------------- constants
B, T, H, W = 4, 22, 256, 256
N_CORES = 8
T_INT = 10
T_HALO = T_INT + 2
DX = 10.0 / 256.0
F32 = mybir.dt.float32
BF = mybir.dt.bfloat16
FP8 = mybir.dt.float8e4
AL = mybir.AluOpType
AF = mybir.ActivationFunctionType
DR = mybir.MatmulPerfMode.DoubleRow

_CACHE = {}


# ------------------------------------------------------- stencil matrices
def _circ_pair(taps: dict) -> np.ndarray:
    """[2, 128, 128] (main.T, corr.T) lhsT blocks for the periodic row
    stencil out[h] = sum_s taps[s] * f[(h+s) % 256], rows packed 2x128."""
    M = np.zeros((256, 256), np.float64)
    for s, a in taps.items():
        for h in range(256):
            M[h, (h + s) % 256] += a
    A = M[:128, :128]
    C = M[:128, 128:256]
    assert np.allclose(M[128:, 128:], A) and np.allclose(M[128:, :128], C)
    return np.stack([A.T, C.T]).astype(np.float32)


def _pair_diag(w0: float, w1: float) -> np.ndarray:
    I = np.eye(128, dtype=np.float32)
    return np.stack([w0 * I, w1 * I])


def _weights() -> np.ndarray:
    mats = [
        _circ_pair({1: 1.0, -1: -1.0}),    # 0 dvy-y
        _circ_pair({0: 1.0, 1: 1.0}),      # 1 vcn
        _circ_pair({0: 3.0, -1: -1.0}),    # 2 gp
        _circ_pair({1: 3.0, 2: -1.0}),     # 3 gm
        _circ_pair({0: 1.0, -1: -1.0}),    # 4 s5
        _circ_pair({1: -2.0, 0: 8.0, -1: -2.0}),  # 5 s4' (y-lap + x-lap centre)
        _pair_diag(1.0, -1.0),             # 6 (u+1,u-1) dvy-x / convx-p
        _pair_diag(-1.0, 1.0),             # 7 convx-q
        _pair_diag(1.0, 1.0),              # 8 uc2
        _pair_diag(3.0, -1.0),             # 9 gxp / gxm
        _pair_diag(0.15625, -0.15625),     # 10 dfdt (DX/DT)
        _pair_diag(-2.0, -2.0),            # 11 x-lap sides
        _pair_diag(-0.5, -0.5),            # 12 pd
    ]
    # [13, 2, 128, 128] (w, i, k, m) -> [128(k), 13, 2, 128(m)]
    return np.ascontiguousarray(np.transpose(np.stack(mats), (2, 0, 1, 3)))


NW = 13


# ------------------------------------------------------------ AP helpers
def _iview_swap(a3: bass.AP, j: int) -> bass.AP:
    """[128, 2(i), 256] ifmap for out-block j from a [128, 2, 256] frame
    view: i=0 -> block j, i=1 -> block 1-j."""
    a = [list(x) for x in a3.ap]
    assert len(a) == 3 and a[1][1] == 2
    sj = a[1][0]
    return bass.AP(tensor=a3.tensor, offset=a3.offset + j * sj,
                   ap=[a[0], [(1 - 2 * j) * sj, 2], a[2]])


def _pack2(v0: bass.AP, v1: bass.AP) -> bass.AP:
    """[128, 2(i), 2(j), 256] ifmap from two same-shape [128, 2, 256]
    views of one tensor (i-dim = the pack pair)."""
    assert v0.tensor is v1.tensor
    a = [list(x) for x in v0.ap]
    assert len(a) == 3 and a[1][1] == 2
    di = v1.offset - v0.offset
    return bass.AP(tensor=v0.tensor, offset=v0.offset,
                   ap=[a[0], [di, 2], a[1], a[2]])


# ------------------------------------------------------------ graph build
def _build():
    nc = bacc.Bacc("TRN2")
    u8e = nc.declare_dram_parameter("u8", [128, T_INT, 2, 260], FP8, isOutput=False)
    v8e = nc.declare_dram_parameter("v8", [128, T_INT, 2, 2, 256], FP8, isOutput=False)
    c8e = nc.declare_dram_parameter("c8", [128, T_HALO, 2, 2, 260], FP8, isOutput=False)
    d8e = nc.declare_dram_parameter("d8", [128, T_HALO, 2, 2, 260], FP8, isOutput=False)
    cbe = nc.declare_dram_parameter("c8b", [128, T_INT, 2, 2, 260], FP8, isOutput=False)
    dbe = nc.declare_dram_parameter("d8b", [128, T_INT, 2, 2, 260], FP8, isOutput=False)
    swe = nc.declare_dram_parameter("stw", [128, NW, 2, 128], FP8, isOutput=False)
    out_ext = nc.declare_dram_parameter("out", [1, 4], F32, isOutput=True)

    with TileContext(nc) as tc:
        with (
            tc.tile_pool(name="const", bufs=1) as constp,
            tc.tile_pool(name="frames", bufs=1) as framesp,
            tc.tile_pool(name="scr", bufs=12) as scr,
            tc.tile_pool(name="psum", bufs=1, space="PSUM") as psp,
        ):
            stw = constp.tile([128, NW, 2, 128], FP8, name="stw_sb")
            nc.sync.dma_start(out=stw, in_=swe[:, :, :, :])

            ones = constp.tile([128, 1], F32, name="ones_sb")
            nc.vector.memset(ones, 1.0)

            accs = constp.tile([128, 3, T_INT], F32, name="accs")

            u8 = framesp.tile([128, T_INT, 2, 260], FP8, name="u8")
            v8 = framesp.tile([128, T_INT, 2, 2, 256], FP8, name="v8")
            c8 = framesp.tile([128, T_HALO, 2, 2, 260], FP8, name="c8")
            d8 = framesp.tile([128, T_HALO, 2, 2, 260], FP8, name="d8")
            c8b = framesp.tile([128, T_INT, 2, 2, 260], FP8, name="c8b")
            d8b = framesp.tile([128, T_INT, 2, 2, 260], FP8, name="d8b")
            ucpM = framesp.tile([128, T_INT, 2, 256], BF, name="ucpM")
            ucmM = framesp.tile([128, T_INT, 2, 256], BF, name="ucmM")
            vcnM = framesp.tile([128, T_INT, 2, 256], BF, name="vcnM")
            divM = framesp.tile([128, T_INT, 2, 256], BF, name="divM")
            sqd = framesp.tile([128, T_INT, 2, 256], BF, name="sqd")

            # DMA waves: tiny first wave so compute starts early
            WAVES = [(0, 1, 0, 3), (1, 4, 3, 5), (5, 5, 8, 4)]
            for (ti, ni, th, nh) in WAVES:
                nc.sync.dma_start(out=u8[:, ti:ti + ni], in_=u8e[:, ti:ti + ni])
                nc.sync.dma_start(out=v8[:, ti:ti + ni], in_=v8e[:, ti:ti + ni])
                nc.sync.dma_start(out=c8[:, th:th + nh], in_=c8e[:, th:th + nh])
                nc.sync.dma_start(out=d8[:, th:th + nh], in_=d8e[:, th:th + nh])
                nc.sync.dma_start(out=c8b[:, ti:ti + ni], in_=cbe[:, ti:ti + ni])
                nc.sync.dma_start(out=d8b[:, ti:ti + ni], in_=dbe[:, ti:ti + ni])

            def ev(tile, t, k=0):
                # even frame view [128, 2, 256] at col offset k (halo'd tile)
                return tile[:, t, :, 2 + k:258 + k]

            def evd(tile, t, k=0):
                # normal-copy frame view of a dup-swapped tile
                return tile[:, t, 0, :, 2 + k:258 + k]

            def stencil(ps3, w, rhs3, start, stop):
                # y-stencil DR pair: out_j = main @ f_j + corr @ f_{1-j}
                for j in (0, 1):
                    nc.tensor.matmul(ps3[:, j, :], stw[:, w],
                                     _iview_swap(rhs3, j),
                                     start=start and j == 0,
                                     stop=stop and j == 1,
                                     skip_group_check=True,
                                     perf_mode=DR)

            def stencil_h(ps3, w, tile, t, start, stop, k=0, halo=True):
                # y-stencil on a host-dup tile: one DR matmul, ifmap
                # [p][v=(normal,swapped)][j][col]
                c0 = 2 + k if halo else 0
                nc.tensor.matmul(ps3[:, :, :], stw[:, w],
                                 tile[:, t, :, :, c0:c0 + 256],
                                 start=start, stop=stop,
                                 skip_group_check=True, perf_mode=DR)

            def pack(ps3, w, v0, v1, start, stop):
                # x/t identity pack: out = w0 * v0 + w1 * v1 (N=512 DR)
                nc.tensor.matmul(ps3[:, :, :], stw[:, w], _pack2(v0, v1),
                                 start=start, stop=stop,
                                 skip_group_check=True, perf_mode=DR)

            STT = nc.vector.scalar_tensor_tensor
            TT = nc.vector.tensor_tensor

            def shared(t):
                uc2p = psp.tile([128, 2, 256], F32, name="uc2p", tag="aux",
                                bufs=3)
                pack(uc2p, 8, ev(u8, t, 0), ev(u8, t, 1), True, True)
                nc.scalar.activation(ucpM[:, t], uc2p, AF.Relu, scale=0.5)
                nc.scalar.activation(ucmM[:, t], uc2p, AF.Relu, scale=-0.5)

                vcnp = psp.tile([128, 2, 256], F32, name="vcnp", tag="aux",
                                bufs=3)
                stencil_h(vcnp, 1, v8, t, True, True, halo=False)
                nc.scalar.mul(vcnM[:, t], vcnp, 0.5)

                dvyp = psp.tile([128, 2, 256], F32, name="dvyp", tag="aux",
                                bufs=3)
                stencil_h(dvyp, 0, v8, t, True, False, halo=False)
                pack(dvyp, 6, ev(u8, t, 1), ev(u8, t, -1), False, True)
                nc.scalar.copy(divM[:, t], dvyp)

            def fields(t):
                for fi, (f8, fb8) in enumerate(((c8, c8b), (d8, d8b))):
                    sfx = f"{'cd'[fi]}{t}"
                    gp = psp.tile([128, 2, 256], F32, name="gp", tag="g",
                                  bufs=3)
                    stencil_h(gp, 2, f8, t + 1, True, True)
                    gm = psp.tile([128, 2, 256], F32, name="gm", tag="g",
                                  bufs=3)
                    stencil_h(gm, 3, f8, t + 1, True, True)
                    p28 = scr.tile([128, 2, 256], FP8, name="p28", tag="p2")
                    STT(p28, vcnM[:, t], 0.0, gp, AL.max, AL.mult)
                    q28 = scr.tile([128, 2, 256], FP8, name="q28", tag="q2")
                    STT(q28, vcnM[:, t], 0.0, gm, AL.min, AL.mult)

                    gxp = psp.tile([128, 2, 256], F32, name="gxp", tag="g",
                                   bufs=3)
                    pack(gxp, 9, evd(f8, t + 1, 0), evd(f8, t + 1, -1),
                         True, True)
                    gxm = psp.tile([128, 2, 256], F32, name="gxm", tag="g",
                                   bufs=3)
                    pack(gxm, 9, evd(f8, t + 1, 1), evd(f8, t + 1, 2),
                         True, True)
                    p8 = scr.tile([128, 2, 258], FP8, name="p8", tag="p8")
                    TT(p8[:, :, 1:257], ucpM[:, t], gxp, AL.mult)
                    nc.gpsimd.tensor_copy(p8[:, :, 0:1], p8[:, :, 256:257])
                    q8 = scr.tile([128, 2, 258], FP8, name="q8", tag="q8")
                    TT(q8[:, :, 1:257], ucmM[:, t], gxm, AL.mult)
                    nc.gpsimd.tensor_copy(q8[:, :, 0:1], q8[:, :, 256:257])

                    pd8 = scr.tile([128, 2, 256], FP8, name="pd8", tag="pd")
                    nc.gpsimd.tensor_tensor(pd8, evd(f8, t + 1), divM[:, t],
                                            AL.mult)

                    t2p = psp.tile([128, 2, 256], F32, name="t2p", tag="t2",
                                   bufs=2)
                    pack(t2p, 10, evd(f8, t + 2), evd(f8, t), True, False)
                    stencil_h(t2p, 5, fb8, t, False, False)
                    pack(t2p, 11, evd(fb8, t, 1), evd(fb8, t, -1),
                         False, False)
                    stencil(t2p, 4, p28[:, :, :], False, False)
                    stencil(t2p, 4, q28[:, :, :], False, False)
                    pack(t2p, 6, p8[:, :, 1:257], p8[:, :, 0:256],
                         False, False)
                    pack(t2p, 7, q8[:, :, 1:257], q8[:, :, 0:256],
                         False, False)
                    pack(t2p, 12, pd8[:, :, :], pd8[:, :, :], False, True)
                    sq = scr.tile([128, 2, 256], BF, name="sq", tag="sq")
                    nc.scalar.activation(sq, t2p, AF.Square,
                                         accum_out=accs[:, 1 + fi, t:t + 1])
                if t % 2 == 1:
                    # continuity loss for the frame pair (t-1, t)
                    k = t // 2
                    nc.scalar.activation(
                        sqd[:, t - 1:t + 1].rearrange("p t j x -> p (t j x)"),
                        divM[:, t - 1:t + 1].rearrange("p t j x -> p (t j x)"),
                        AF.Square, accum_out=accs[:, 0, k:k + 1])

            # shared chain runs one frame ahead of the field chains
            shared(0)
            for t in range(T_INT):
                if t + 1 < T_INT:
                    shared(t + 1)
                fields(t)

            # ---------------- final reduction to [1, 3]
            red3 = constp.tile([128, 3], F32, name="red3")
            nc.vector.tensor_reduce(red3[:, 0:1], accs[:, 0, 0:5],
                                    mybir.AxisListType.X, AL.add)
            for k in (1, 2):
                nc.vector.tensor_reduce(red3[:, k:k + 1], accs[:, k, :],
                                        mybir.AxisListType.X, AL.add)
            psr = psp.tile([1, 4], F32, name="psr", tag="aux", bufs=3)
            nc.tensor.matmul(psr[:, 0:3], ones, red3, start=True, stop=True)
            outt = constp.tile([1, 4], F32, name="outt")
            nc.vector.memset(outt, 0.0)
            nc.scalar.copy(outt[:, 0:3], psr[:, 0:3])
            nc.sync.dma_start(out=out_ext[:, :], in_=outt)

    nc.compile()
    return nc


def _get_nc():
    if "nc" not in _CACHE:
        _CACHE["nc"] = _build()
        _CACHE["stw"] = _weights()
    return _CACHE["nc"]


def _prepack(x, halo, dup=False):
    """[T', 256, 256] fp32 -> fp8 [128, T', (2,) 2, 260 or 256] SBUF layout.

    dup=True adds a block-swapped copy (dim 2) so y-stencils with wrap
    correction are a single DoubleRow matmul over an affine ifmap."""
    import ml_dtypes
    t = x.shape[0]
    y = x.reshape(t, 2, 128, 256).transpose(2, 0, 1, 3)
    if halo:
        y = np.concatenate([y[..., 254:256], y, y[..., 0:2]], axis=-1)
    if dup:
        y = np.stack([y, y[:, :, ::-1]], axis=2)
    return np.ascontiguousarray(y.astype(ml_dtypes.float8_e4m3))


def _make_in_maps(u, v, c, Dxx):
    import ml_dtypes
    u = np.asarray(u, dtype=np.float32)
    v = np.asarray(v, dtype=np.float32)
    c = np.asarray(c, dtype=np.float32)
    d = np.asarray(Dxx, dtype=np.float32)
    stw = np.ascontiguousarray(_CACHE["stw"].astype(ml_dtypes.float8_e4m3))
    in_maps = []
    for i in range(N_CORES):
        b, h = i // 2, i % 2
        t0 = 1 + T_INT * h
        in_maps.append({
            "u8": _prepack(u[b, t0:t0 + T_INT], True),
            "v8": _prepack(v[b, t0:t0 + T_INT], False, dup=True),
            "c8": _prepack(c[b, t0 - 1:t0 + T_INT + 1], True, dup=True),
            "d8": _prepack(d[b, t0 - 1:t0 + T_INT + 1], True, dup=True),
            "c8b": _prepack(1.28 * c[b, t0:t0 + T_INT], True, dup=True),
            "d8b": _prepack(1.28 * d[b, t0:t0 + T_INT], True, dup=True),
            "stw": stw,
        })
    return in_maps


def _combine(results):
    s = np.zeros(3, dtype=np.float64)
    for r in results:
        s += np.asarray(r["out"], dtype=np.float64)[0, :3]
    n = B * (T - 2) * H * W
    scale = 1.0 / (4.0 * DX * DX * n)
    loss_cont = scale * s[0]
    loss_conc = scale * s[1]
    loss_dxx = scale * s[2]
    total = loss_cont + loss_conc + loss_dxx
    return np.array([total, loss_cont, loss_conc, loss_dxx], dtype=np.float32)


def kernel(u, v, c, Dxx):
    nc = _get_nc()
    in_maps = _make_in_maps(u, v, c, Dxx)
    last_err = None
    for _attempt in range(3):
        try:
            res = run_bass_kernel_spmd(nc, in_maps,
                                       core_ids=list(range(N_CORES)))
            return _combine(res.results)
        except Exception as e:  # transient NRT/device hiccups: retry
            last_err = e
    raise last_err


if __name__ == "__main__":
    rng = np.random.default_rng(0)
    inputs = {
        "u": rng.standard_normal((B, T, H, W), dtype=np.float32),
        "v": rng.standard_normal((B, T, H, W), dtype=np.float32),
        "c": rng.random((B, T, H, W), dtype=np.float32),
        "Dxx": rng.random((B, T, H, W), dtype=np.float32),
    }
    print(kernel(**inputs))
